# revision 31
# baseline (speedup 1.0000x reference)
"""AttentionBlock (GroupNorm + MHA + out-proj + residual) on 8 trn2 NeuronCores.

Data-parallel over batch: B=16 -> 2 batches per core. Inside each core:
  x[b] : [512, 1024]  (C x HW), channel chunks of 128 on partitions.
  GroupNorm(8 groups of 64 ch) -> xn (stats via mask-matmuls on the PE)
  q,k = W_qk @ xn in [ch, tok] layout; v built directly transposed [tok, ch]
  via xn-as-lhsT so the PV matmul needs no transposes.
  per head h (d=64): St[j,i] = k_h^T q_h ; P = exp(St/8) (ScalarE, fused scale,
  one activation per two j-chunks straight out of PSUM)
  PV with ones-augmented v^T -> O_un[d,i] rows 0..63, softmax sums in row 64
  normalize via reciprocal + DRAM-bounce broadcast + multiply (GPSIMD)
  y = W_out @ O + b_out_eff + x   (v-bias folded into b_out_eff on host)

All big matmuls run in float32r. QKV chunk GEMMs are interleaved into the
(ScalarE-bound) attention stream per head-pair to keep the PE warm.
"""
import os
import sys

sys.path.insert(0, "/opt/trn_rl_repo")

import numpy as np

import concourse.bass as bass
import concourse.tile as tile
from concourse import bacc, mybir
from concourse import bass_utils

F32 = mybir.dt.float32
F32R = mybir.dt.float32r
AF = mybir.ActivationFunctionType
OP = mybir.AluOpType
X = mybir.AxisListType.X

NB = 2          # batches per core
C = 512         # channels
HW = 1024       # tokens
NH = 8          # heads
NG = 8          # groups
NC_CH = 4       # channel chunks of 128
EPS = 1e-5
GSIZE = C // NG * HW  # elements per group = 65536


def build_program(nc, tc, ctx):
    x_d = nc.dram_tensor("x", [NB, C, HW], F32, kind="ExternalInput").ap()
    wt_d = nc.dram_tensor("wt", [C, 3 * C], F32R, kind="ExternalInput").ap()
    wto_d = nc.dram_tensor("wto", [C, C], F32R, kind="ExternalInput").ap()
    bq_d = nc.dram_tensor("bq", [128, 8], F32, kind="ExternalInput").ap()
    gam_d = nc.dram_tensor("gam", [128, NC_CH], F32, kind="ExternalInput").ap()
    bet_d = nc.dram_tensor("bet", [128, NC_CH], F32, kind="ExternalInput").ap()
    bout_d = nc.dram_tensor("bout", [128, NC_CH], F32, kind="ExternalInput").ap()
    gmask_d = nc.dram_tensor("gmask", [128, NC_CH, 2 * NG], F32, kind="ExternalInput").ap()
    gmaskT_d = nc.dram_tensor("gmaskT", [2 * NG, NC_CH, 128], F32, kind="ExternalInput").ap()
    vtones_d = nc.dram_tensor("vtones", [128, NH, NH, 1], F32R, kind="ExternalInput").ap()
    y_d = nc.dram_tensor("y", [NB, C, HW], F32, kind="ExternalOutput").ap()

    consts = ctx.enter_context(tc.tile_pool(name="consts", bufs=1))
    x_pool = ctx.enter_context(tc.tile_pool(name="x", bufs=2))
    xn_pool = ctx.enter_context(tc.tile_pool(name="xn", bufs=1))
    qk_pool = ctx.enter_context(tc.tile_pool(name="qk", bufs=1))
    vt_pool = ctx.enter_context(tc.tile_pool(name="vt", bufs=1))
    o_pool = ctx.enter_context(tc.tile_pool(name="o", bufs=1))
    p_pool = ctx.enter_context(tc.tile_pool(name="p", bufs=2))
    ou_pool = ctx.enter_context(tc.tile_pool(name="ou", bufs=2))
    y_pool = ctx.enter_context(tc.tile_pool(name="y", bufs=2))
    gn_pool = ctx.enter_context(tc.tile_pool(name="gn", bufs=2))
    r_pool = ctx.enter_context(tc.tile_pool(name="r", bufs=3))
    rb_pool = ctx.enter_context(tc.tile_pool(name="rb", bufs=1))
    sc_pool = ctx.enter_context(tc.tile_pool(name="sc", bufs=1))
    dram = ctx.enter_context(tc.tile_pool(name="dram", bufs=2, space="DRAM"))
    # PSUM: 8 banks = st(4) + o(2) + gp(2)
    ps_st_pool = ctx.enter_context(tc.tile_pool(name="ps_st", bufs=1, space="PSUM"))
    ps_o_pool = ctx.enter_context(tc.tile_pool(name="ps_o", bufs=1, space="PSUM"))
    ps_gp_pool = ctx.enter_context(tc.tile_pool(name="ps_gp", bufs=1, space="PSUM"))

    # ---- constants (gmask first: it feeds the PE warm-up) ----
    gmask = consts.tile([128, NC_CH, 2 * NG], F32)
    nc.sync.dma_start(gmask[:], gmask_d)
    gmaskT = consts.tile([2 * NG, NC_CH, 128], F32)
    nc.sync.dma_start(gmaskT[:], gmaskT_d)
    bq_sb = consts.tile([128, 8], F32)
    nc.sync.dma_start(bq_sb[:], bq_d)
    gam_sb = consts.tile([128, NC_CH], F32)
    nc.sync.dma_start(gam_sb[:], gam_d)
    bet_sb = consts.tile([128, NC_CH], F32)
    nc.sync.dma_start(bet_sb[:], bet_d)
    bout_sb = consts.tile([128, NC_CH], F32)
    nc.sync.dma_start(bout_sb[:], bout_d)
    wt_sb = consts.tile([128, NC_CH, 3 * C], F32R)
    nc.sync.dma_start(wt_sb[:], wt_d.rearrange("(c p) o -> p c o", p=128))
    wto_sb = consts.tile([128, NC_CH, C], F32R)
    nc.sync.dma_start(wto_sb[:], wto_d.rearrange("(c p) o -> p c o", p=128))

    # v^T tiles with the ones column (col 64 of each head's 65-col block)
    vt_sb = vt_pool.tile([128, NH, NH * 65], F32R)  # [j-part, j-chunk, h*65+e]
    vt_v = vt_sb.rearrange("p j (h e) -> p j h e", e=65)
    nc.sync.dma_start(vt_v[:, :, :, 64:65], vtones_d)

    # ---- PE warm-up: ~5us of junk matmuls so HAM reaches K=8/8 before the
    # real pipeline starts (and stays there). Reads gmask (first DMA in).
    wu_ps = ps_gp_pool.tile([128, HW], F32, tag="gp")
    gm_flat = gmask.rearrange("p c g -> p (c g)")
    for w in range(48):
        nc.tensor.matmul(
            wu_ps[0 : 2 * NG, 0:64], gmask[:, 0, :], gm_flat[:],
            start=True, stop=True,
        )

    def load_x(b):
        x_sb = x_pool.tile([128, NC_CH, HW], F32)
        for c in range(NC_CH):
            nc.sync.dma_start(
                x_sb[:, c, :], x_d[b, c * 128 : (c + 1) * 128, :]
            )
        return x_sb

    def groupnorm(b, x_sb, xn_sb):
        stats2 = gn_pool.tile([128, NC_CH, 2], F32)
        scratch = sc_pool.tile([128, HW], F32)
        for c in range(NC_CH):
            # row sums on DVE, row sum-of-squares on ScalarE (Square + accum),
            # so the two passes run on different engines in parallel.
            nc.vector.reduce_sum(stats2[:, c, 0:1], x_sb[:, c, :], axis=X)
            nc.scalar.activation(
                scratch[:], x_sb[:, c, :], AF.Square,
                accum_out=stats2[:, c, 1:2],
            )
        gstat_ps = ps_gp_pool.tile([128, HW], F32, tag="gp")
        for c in range(NC_CH):
            nc.tensor.matmul(
                gstat_ps[0 : 2 * NG, 0:2],
                gmask[:, c, :],
                stats2[:, c, :],
                start=(c == 0),
                stop=(c == NC_CH - 1),
            )
        # [16, 2] (sum, sumsq) -> msr [16, 2] = (mean, rstd)
        msr = gn_pool.tile([2 * NG, 2], F32)
        var = gn_pool.tile([2 * NG, 1], F32)
        nc.vector.tensor_scalar_mul(msr[:], gstat_ps[0 : 2 * NG, 0:2], 1.0 / GSIZE)
        nc.vector.tensor_tensor(var[:], msr[:, 0:1], msr[:, 0:1], op=OP.mult)
        nc.vector.tensor_tensor(var[:], msr[:, 1:2], var[:], op=OP.subtract)
        nc.vector.tensor_scalar_add(var[:], var[:], EPS)
        nc.vector.reciprocal(var[:], var[:])
        nc.scalar.activation(msr[:, 1:2], var[:], AF.Sqrt)  # rstd = sqrt(1/(var+eps))

        bc_ps = ps_gp_pool.tile([128, HW], F32, tag="gp")
        for c in range(NC_CH):
            nc.tensor.matmul(
                bc_ps[:, 2 * c : 2 * c + 2],
                gmaskT[:, c, :],
                msr[:],
                start=True,
                stop=True,
            )
        ab = gn_pool.tile([128, NC_CH, 2], F32)
        tmp = gn_pool.tile([128, 1], F32)
        for c in range(NC_CH):
            # a = rstd_bc * gamma ; b2 = beta - mean_bc * a
            nc.vector.tensor_tensor(
                ab[:, c, 0:1], bc_ps[:, 2 * c + 1 : 2 * c + 2],
                gam_sb[:, c : c + 1], op=OP.mult,
            )
            nc.vector.tensor_tensor(
                tmp[:], bc_ps[:, 2 * c : 2 * c + 1], ab[:, c, 0:1], op=OP.mult
            )
            nc.vector.tensor_tensor(
                ab[:, c, 1:2], bet_sb[:, c : c + 1], tmp[:], op=OP.subtract
            )
        for c in range(NC_CH):
            nc.vector.tensor_scalar(
                out=xn_sb[:, c, :],
                in0=x_sb[:, c, :],
                scalar1=ab[:, c, 0:1],
                scalar2=ab[:, c, 1:2],
                op0=OP.mult,
                op1=OP.add,
            )

    def v_transposed(b, xn_sb):
        # v^T: [tok, ch] layout via xn as lhsT (no bias: folded into b_out)
        for jp in range(4):
            ps_vt = ps_gp_pool.tile([128, HW], F32, tag="gp")
            for j2 in range(2):
                j = 2 * jp + j2
                for c in range(NC_CH):
                    nc.tensor.matmul(
                        ps_vt[:, j2 * 512 : (j2 + 1) * 512],
                        xn_sb[:, c, j * 128 : (j + 1) * 128],
                        wt_sb[:, c, 2 * C : 3 * C],
                        start=(c == 0),
                        stop=(c == NC_CH - 1),
                    )
            nc.vector.tensor_copy(
                vt_v[:, 2 * jp : 2 * jp + 2, :, 0:64],
                ps_vt[:].rearrange("p (j h e) -> p j h e", j=2, h=NH),
            )

    def qk_chunk(b, xn_sb, qk_sb, m):
        # q,k output channels m*128..(m+1)*128 in [ch, tok] layout
        ps_qk = ps_gp_pool.tile([128, HW], F32, tag="gp")
        for c in range(NC_CH):
            for half in range(2):
                nc.tensor.matmul(
                    ps_qk[:, half * 512 : (half + 1) * 512],
                    wt_sb[:, c, m * 128 : (m + 1) * 128],
                    xn_sb[:, c, half * 512 : (half + 1) * 512],
                    start=(c == 0),
                    stop=(c == NC_CH - 1),
                )
        nc.vector.tensor_scalar_add(qk_sb[:, m, :], ps_qk[:], bq_sb[:, m : m + 1])

    def attn_head(b, qk_sb, o_sb, h):
        po = 64 * (h % 2)
        q_ap = qk_sb[po : po + 64, h // 2, :]
        k_ap = qk_sb[po : po + 64, 4 + h // 2, :]
        ps_o = ps_o_pool.tile([128, HW], F32, tag="po")
        for sj in range(4):  # super-chunks of 2 j-chunks
            ps_st = ps_st_pool.tile([128, 2 * HW], F32, tag="st")
            for jj in range(2):
                j = 2 * sj + jj
                for half in range(2):
                    nc.tensor.matmul(
                        ps_st[:, jj * HW + half * 512 : jj * HW + (half + 1) * 512],
                        k_ap[:, j * 128 : (j + 1) * 128],
                        q_ap[:, half * 512 : (half + 1) * 512],
                        start=True,
                        stop=True,
                    )
            p_t = p_pool.tile([128, 2 * HW], F32R, tag="p_t")
            nc.scalar.activation(p_t[:], ps_st[:], AF.Exp, scale=0.125)
            for jj in range(2):
                j = 2 * sj + jj
                for half in range(2):
                    nc.tensor.matmul(
                        ps_o[0:65, half * 512 : (half + 1) * 512],
                        vt_sb[:, j, 65 * h : 65 * h + 65],
                        p_t[:, jj * HW + half * 512 : jj * HW + (half + 1) * 512],
                        start=(sj == 0 and jj == 0),
                        stop=(sj == 3 and jj == 1),
                    )
        # early-drain PSUM -> SBUF so the next head's PV can start; the
        # normalization chain then runs off the critical path.
        o_un = ou_pool.tile([64, HW], F32)
        nc.vector.tensor_copy(o_un[:], ps_o[0:64, :])
        s_row = r_pool.tile([1, HW], F32, tag="row")
        nc.vector.tensor_copy(s_row[:], ps_o[64:65, :])
        # normalize: O[d,i] * (1/s[i]); sums broadcast via DRAM bounce.
        r_row = r_pool.tile([1, HW], F32, tag="row")
        nc.vector.reciprocal_approx_fast(r_row[:], s_row[:])
        dr = dram.tile([1, HW], F32)
        nc.sync.dma_start(dr[:], r_row[:])
        rb = rb_pool.tile([64, HW], F32)
        nc.sync.dma_start(rb[:], dr[:].to_broadcast((64, HW)))
        use_gs = os.environ.get("K_NORM_GS", "1") == "1" and h < NH - 1
        norm_eng = nc.gpsimd if use_gs else nc.vector
        norm_eng.tensor_tensor(
            o_sb[po : po + 64, h // 2, :], o_un[:], rb[:], op=OP.mult
        )

    def outproj(b, o_sb, x_sb):
        for r in range(NC_CH):
            ps_y = ps_gp_pool.tile([128, HW], F32, tag="gp")
            for c in range(NC_CH):
                for half in range(2):
                    nc.tensor.matmul(
                        ps_y[:, half * 512 : (half + 1) * 512],
                        wto_sb[:, c, r * 128 : (r + 1) * 128],
                        o_sb[:, c, half * 512 : (half + 1) * 512],
                        start=(c == 0),
                        stop=(c == NC_CH - 1),
                    )
            y_t = y_pool.tile([128, HW], F32)
            nc.vector.scalar_tensor_tensor(
                out=y_t[:],
                in0=ps_y[:],
                scalar=bout_sb[:, r : r + 1],
                in1=x_sb[:, r, :],
                op0=OP.add,
                op1=OP.add,
            )
            nc.sync.dma_start(y_d[b, r * 128 : (r + 1) * 128, :], y_t[:])

    prev = None
    for b in range(NB):
        x_sb = load_x(b)
        xn_sb = xn_pool.tile([128, NC_CH, HW], F32R)
        qk_sb = qk_pool.tile([128, 8, HW], F32R)
        o_sb = o_pool.tile([128, NC_CH, HW], F32R)
        groupnorm(b, x_sb, xn_sb)
        v_transposed(b, xn_sb)
        for p in range(4):  # head pairs; qk chunks arrive just-in-time
            qk_chunk(b, xn_sb, qk_sb, p)
            qk_chunk(b, xn_sb, qk_sb, 4 + p)
            if p == 0 and prev is not None:
                outproj(*prev)
            attn_head(b, qk_sb, o_sb, 2 * p)
            attn_head(b, qk_sb, o_sb, 2 * p + 1)
        prev = (b, o_sb, x_sb)
    outproj(*prev)


_NC_CACHE = None


def _build():
    global _NC_CACHE
    if _NC_CACHE is not None:
        return _NC_CACHE
    import contextlib

    nc = bacc.Bacc("TRN2", target_bir_lowering=False, debug=False)
    with tile.TileContext(nc) as tc:
        with contextlib.ExitStack() as ctx:
            build_program(nc, tc, ctx)
    nc.compile()
    _NC_CACHE = nc
    return nc


def make_in_maps(x, gamma, beta, w_qkv, b_qkv, w_out, b_out):
    x = np.ascontiguousarray(np.asarray(x, dtype=np.float32))
    gamma = np.asarray(gamma, dtype=np.float32)
    beta = np.asarray(beta, dtype=np.float32)
    w_qkv = np.asarray(w_qkv, dtype=np.float32)
    b_qkv = np.asarray(b_qkv, dtype=np.float32)
    w_out = np.asarray(w_out, dtype=np.float32)
    b_out = np.asarray(b_out, dtype=np.float32)

    B, Cc, H, W = x.shape
    assert (B, Cc, H, W) == (16, 512, 32, 32)

    # host-side weight layout transforms (pure layout; no compute moved
    # off-device except the exact fold of the v-bias: softmax rows sum to 1,
    # so attn @ (v + b_v 1^T) = attn @ v + b_v, and W_out @ b_v folds into b_out)
    wt = np.ascontiguousarray(w_qkv.T)                      # [512, 1536]
    wto = np.ascontiguousarray(w_out.T)                     # [512, 512]
    b_out_eff = b_out + w_out @ b_qkv[2 * C : 3 * C]
    bq = np.ascontiguousarray(b_qkv[: 2 * C].reshape(8, 128).T)   # [128, 8]
    gam = np.ascontiguousarray(gamma.reshape(NC_CH, 128).T)       # [128, 4]
    bet = np.ascontiguousarray(beta.reshape(NC_CH, 128).T)
    bout = np.ascontiguousarray(b_out_eff.reshape(NC_CH, 128).T)

    gmask_np = np.zeros((128, NC_CH, 2 * NG), dtype=np.float32)
    gmaskT_np = np.zeros((2 * NG, NC_CH, 128), dtype=np.float32)
    for c in range(NC_CH):
        gmask_np[0:64, c, 2 * c] = 1.0
        gmask_np[64:128, c, 2 * c + 1] = 1.0
        gmaskT_np[2 * c, c, 0:64] = 1.0
        gmaskT_np[2 * c + 1, c, 64:128] = 1.0

    xr = x.reshape(16, 512, 1024)
    in_maps = []
    for core in range(8):
        in_maps.append(
            {
                "x": np.ascontiguousarray(xr[2 * core : 2 * core + 2]),
                "wt": wt,
                "wto": wto,
                "bq": bq,
                "gam": gam,
                "bet": bet,
                "bout": bout,
                "gmask": gmask_np,
                "gmaskT": gmaskT_np,
                "vtones": np.ones((128, NH, NH, 1), dtype=np.float32),
            }
        )
    return in_maps


def kernel(x, gamma, beta, w_qkv, b_qkv, w_out, b_out):
    in_maps = make_in_maps(x, gamma, beta, w_qkv, b_qkv, w_out, b_out)
    nc = _build()
    res = bass_utils.run_bass_kernel_spmd(nc, in_maps, core_ids=list(range(8)))
    out = np.concatenate([r["y"] for r in res.results], axis=0)
    return out.reshape(16, 512, 32, 32).astype(np.float32)


# revision 32
# speedup vs baseline: 1.3920x; 1.3920x over previous
"""AttentionBlock (GroupNorm + MHA + out-proj + residual) on 8 trn2 NeuronCores.

Data-parallel over batch: B=16 -> 2 batches per core. Inside each core:
  x[b] : [512, 1024]  (C x HW), channel chunks of 128 on partitions.
  GroupNorm(8 groups of 64 ch) -> xn (stats via mask-matmuls on the PE)
  q,k = W_qk @ xn in [ch, tok] layout; v built directly transposed [tok, ch]
  via xn-as-lhsT so the PV matmul needs no transposes.
  per head h (d=64): St[j,i] = k_h^T q_h ; P = exp(St/8) (ScalarE, fused scale,
  one activation per two j-chunks straight out of PSUM)
  PV with ones-augmented v^T -> O_un[d,i] rows 0..63, softmax sums in row 64
  normalize via reciprocal + DRAM-bounce broadcast + multiply (GPSIMD)
  y = W_out @ O + b_out_eff + x   (v-bias folded into b_out_eff on host)

All big matmuls run in float32r. QKV chunk GEMMs are interleaved into the
(ScalarE-bound) attention stream per head-pair to keep the PE warm.
"""
import os
import sys

sys.path.insert(0, "/opt/trn_rl_repo")

import numpy as np

import concourse.bass as bass
import concourse.tile as tile
from concourse import bacc, mybir
from concourse import bass_utils

F32 = mybir.dt.float32
F32R = mybir.dt.float32r
AF = mybir.ActivationFunctionType
OP = mybir.AluOpType
X = mybir.AxisListType.X

NB = 2          # batches per core
C = 512         # channels
HW = 1024       # tokens
NH = 8          # heads
NG = 8          # groups
NC_CH = 4       # channel chunks of 128
EPS = 1e-5
GSIZE = C // NG * HW  # elements per group = 65536


def build_program(nc, tc, ctx):
    x_d = nc.dram_tensor("x", [NB, C, HW], F32, kind="ExternalInput").ap()
    wt_d = nc.dram_tensor("wt", [C, 3 * C], F32R, kind="ExternalInput").ap()
    wto_d = nc.dram_tensor("wto", [C, C], F32R, kind="ExternalInput").ap()
    bq_d = nc.dram_tensor("bq", [128, 8], F32, kind="ExternalInput").ap()
    gam_d = nc.dram_tensor("gam", [128, NC_CH], F32, kind="ExternalInput").ap()
    bet_d = nc.dram_tensor("bet", [128, NC_CH], F32, kind="ExternalInput").ap()
    bout_d = nc.dram_tensor("bout", [128, NC_CH], F32, kind="ExternalInput").ap()
    gmask_d = nc.dram_tensor("gmask", [128, NC_CH, 2 * NG], F32, kind="ExternalInput").ap()
    gmaskT_d = nc.dram_tensor("gmaskT", [2 * NG, NC_CH, 128], F32, kind="ExternalInput").ap()
    vtones_d = nc.dram_tensor("vtones", [128, NH, NH, 1], F32R, kind="ExternalInput").ap()
    y_d = nc.dram_tensor("y", [NB, C, HW], F32, kind="ExternalOutput").ap()

    consts = ctx.enter_context(tc.tile_pool(name="consts", bufs=1))
    x_pool = ctx.enter_context(tc.tile_pool(name="x", bufs=2))
    xn_pool = ctx.enter_context(tc.tile_pool(name="xn", bufs=1))
    qk_pool = ctx.enter_context(tc.tile_pool(name="qk", bufs=1))
    vt_pool = ctx.enter_context(tc.tile_pool(name="vt", bufs=1))
    o_pool = ctx.enter_context(tc.tile_pool(name="o", bufs=1))
    p_pool = ctx.enter_context(tc.tile_pool(name="p", bufs=3))
    ou_pool = ctx.enter_context(tc.tile_pool(name="ou", bufs=2))
    y_pool = ctx.enter_context(tc.tile_pool(name="y", bufs=2))
    gn_pool = ctx.enter_context(tc.tile_pool(name="gn", bufs=2))
    r_pool = ctx.enter_context(tc.tile_pool(name="r", bufs=3))
    rb_pool = ctx.enter_context(tc.tile_pool(name="rb", bufs=1))
    sc_pool = ctx.enter_context(tc.tile_pool(name="sc", bufs=1))
    dram = ctx.enter_context(tc.tile_pool(name="dram", bufs=2, space="DRAM"))
    # PSUM: 8 banks = st(4) + o(2) + gp(2)
    ps_st_pool = ctx.enter_context(tc.tile_pool(name="ps_st", bufs=2, space="PSUM"))
    ps_o_pool = ctx.enter_context(tc.tile_pool(name="ps_o", bufs=1, space="PSUM"))
    ps_gp_pool = ctx.enter_context(tc.tile_pool(name="ps_gp", bufs=1, space="PSUM"))

    # ---- constants (gmask first: it feeds the PE warm-up) ----
    gmask = consts.tile([128, NC_CH, 2 * NG], F32)
    nc.sync.dma_start(gmask[:], gmask_d)
    gmaskT = consts.tile([2 * NG, NC_CH, 128], F32)
    nc.sync.dma_start(gmaskT[:], gmaskT_d)
    bq_sb = consts.tile([128, 8], F32)
    nc.sync.dma_start(bq_sb[:], bq_d)
    gam_sb = consts.tile([128, NC_CH], F32)
    nc.sync.dma_start(gam_sb[:], gam_d)
    bet_sb = consts.tile([128, NC_CH], F32)
    nc.sync.dma_start(bet_sb[:], bet_d)
    bout_sb = consts.tile([128, NC_CH], F32)
    nc.sync.dma_start(bout_sb[:], bout_d)
    wt_sb = consts.tile([128, NC_CH, 3 * C], F32R)
    nc.sync.dma_start(wt_sb[:], wt_d.rearrange("(c p) o -> p c o", p=128))
    wto_sb = consts.tile([128, NC_CH, C], F32R)
    nc.sync.dma_start(wto_sb[:], wto_d.rearrange("(c p) o -> p c o", p=128))

    # v^T tiles with the ones column (col 64 of each head's 65-col block)
    vt_sb = vt_pool.tile([128, NH, NH * 65], F32R)  # [j-part, j-chunk, h*65+e]
    vt_v = vt_sb.rearrange("p j (h e) -> p j h e", e=65)
    nc.sync.dma_start(vt_v[:, :, :, 64:65], vtones_d)

    # ---- PE warm-up: ~5us of junk matmuls so HAM reaches K=8/8 before the
    # real pipeline starts (and stays there). Reads gmask (first DMA in).
    wu_ps = ps_gp_pool.tile([128, HW], F32, tag="gp")
    gm_flat = gmask.rearrange("p c g -> p (c g)")
    for w in range(24):
        nc.tensor.matmul(
            wu_ps[0 : 2 * NG, 0:64], gmask[:, 0, :], gm_flat[:],
            start=True, stop=True,
        )

    def load_x(b):
        x_sb = x_pool.tile([128, NC_CH, HW], F32)
        for c in range(NC_CH):
            nc.sync.dma_start(
                x_sb[:, c, :], x_d[b, c * 128 : (c + 1) * 128, :]
            )
            if b == 0:
                # staggered PE warm-ups riding on each x chunk as it lands
                for w in range(6):
                    nc.tensor.matmul(
                        wu_ps[0 : 2 * NG, 0:512],
                        gmask[:, 0, :],
                        x_sb[:, c, 0:512].bitcast(F32),
                        start=True, stop=True,
                    )
        return x_sb

    def groupnorm(b, x_sb, xn_sb):
        stats2 = gn_pool.tile([128, NC_CH, 2], F32)
        scratch = sc_pool.tile([128, HW], F32)
        for c in range(NC_CH):
            # row sums on DVE, row sum-of-squares on ScalarE (Square + accum),
            # so the two passes run on different engines in parallel.
            nc.vector.reduce_sum(stats2[:, c, 0:1], x_sb[:, c, :], axis=X)
            nc.scalar.activation(
                scratch[:], x_sb[:, c, :], AF.Square,
                accum_out=stats2[:, c, 1:2],
            )
        gstat_ps = ps_gp_pool.tile([128, HW], F32, tag="gp")
        for c in range(NC_CH):
            nc.tensor.matmul(
                gstat_ps[0 : 2 * NG, 0:2],
                gmask[:, c, :],
                stats2[:, c, :],
                start=(c == 0),
                stop=(c == NC_CH - 1),
            )
        # [16, 2] (sum, sumsq) -> msr [16, 2] = (mean, rstd)
        msr = gn_pool.tile([2 * NG, 2], F32)
        var = gn_pool.tile([2 * NG, 1], F32)
        nc.vector.tensor_scalar_mul(msr[:], gstat_ps[0 : 2 * NG, 0:2], 1.0 / GSIZE)
        nc.vector.tensor_tensor(var[:], msr[:, 0:1], msr[:, 0:1], op=OP.mult)
        nc.vector.tensor_tensor(var[:], msr[:, 1:2], var[:], op=OP.subtract)
        nc.vector.tensor_scalar_add(var[:], var[:], EPS)
        nc.vector.reciprocal(var[:], var[:])
        nc.scalar.activation(msr[:, 1:2], var[:], AF.Sqrt)  # rstd = sqrt(1/(var+eps))

        bc_ps = ps_gp_pool.tile([128, HW], F32, tag="gp")
        for c in range(NC_CH):
            nc.tensor.matmul(
                bc_ps[:, 2 * c : 2 * c + 2],
                gmaskT[:, c, :],
                msr[:],
                start=True,
                stop=True,
            )
        ab = gn_pool.tile([128, NC_CH, 2], F32)
        tmp = gn_pool.tile([128, 1], F32)
        for c in range(NC_CH):
            # a = rstd_bc * gamma ; b2 = beta - mean_bc * a
            nc.vector.tensor_tensor(
                ab[:, c, 0:1], bc_ps[:, 2 * c + 1 : 2 * c + 2],
                gam_sb[:, c : c + 1], op=OP.mult,
            )
            nc.vector.tensor_tensor(
                tmp[:], bc_ps[:, 2 * c : 2 * c + 1], ab[:, c, 0:1], op=OP.mult
            )
            nc.vector.tensor_tensor(
                ab[:, c, 1:2], bet_sb[:, c : c + 1], tmp[:], op=OP.subtract
            )
        for c in range(NC_CH):
            nc.vector.tensor_scalar(
                out=xn_sb[:, c, :],
                in0=x_sb[:, c, :],
                scalar1=ab[:, c, 0:1],
                scalar2=ab[:, c, 1:2],
                op0=OP.mult,
                op1=OP.add,
            )

    def v_transposed(b, xn_sb):
        # v^T: [tok, ch] layout via xn as lhsT (no bias: folded into b_out)
        for jp in range(4):
            ps_vt = ps_gp_pool.tile([128, HW], F32, tag="gp")
            for j2 in range(2):
                j = 2 * jp + j2
                for c in range(NC_CH):
                    nc.tensor.matmul(
                        ps_vt[:, j2 * 512 : (j2 + 1) * 512],
                        xn_sb[:, c, j * 128 : (j + 1) * 128],
                        wt_sb[:, c, 2 * C : 3 * C],
                        start=(c == 0),
                        stop=(c == NC_CH - 1),
                    )
            nc.vector.tensor_copy(
                vt_v[:, 2 * jp : 2 * jp + 2, :, 0:64],
                ps_vt[:].rearrange("p (j h e) -> p j h e", j=2, h=NH),
            )

    def qk_chunk(b, xn_sb, qk_sb, m):
        # q,k output channels m*128..(m+1)*128 in [ch, tok] layout
        ps_qk = ps_gp_pool.tile([128, HW], F32, tag="gp")
        for c in range(NC_CH):
            for half in range(2):
                nc.tensor.matmul(
                    ps_qk[:, half * 512 : (half + 1) * 512],
                    wt_sb[:, c, m * 128 : (m + 1) * 128],
                    xn_sb[:, c, half * 512 : (half + 1) * 512],
                    start=(c == 0),
                    stop=(c == NC_CH - 1),
                )
        nc.vector.tensor_scalar_add(qk_sb[:, m, :], ps_qk[:], bq_sb[:, m : m + 1])

    def attn_head(b, qk_sb, o_sb, h):
        po = 64 * (h % 2)
        q_ap = qk_sb[po : po + 64, h // 2, :]
        k_ap = qk_sb[po : po + 64, 4 + h // 2, :]
        ps_o = ps_o_pool.tile([128, HW], F32, tag="po")
        for j in range(8):
            ps_st = ps_st_pool.tile([128, HW], F32, tag="st")
            for half in range(2):
                nc.tensor.matmul(
                    ps_st[:, half * 512 : (half + 1) * 512],
                    k_ap[:, j * 128 : (j + 1) * 128],
                    q_ap[:, half * 512 : (half + 1) * 512],
                    start=True,
                    stop=True,
                )
            p_t = p_pool.tile([128, HW], F32R, tag="p_t")
            nc.scalar.activation(p_t[:], ps_st[:], AF.Exp, scale=0.125)
            for half in range(2):
                nc.tensor.matmul(
                    ps_o[0:65, half * 512 : (half + 1) * 512],
                    vt_sb[:, j, 65 * h : 65 * h + 65],
                    p_t[:, half * 512 : (half + 1) * 512],
                    start=(j == 0),
                    stop=(j == 7),
                )
        # early-drain PSUM -> SBUF so the next head's PV can start; the
        # normalization chain then runs off the critical path.
        o_un = ou_pool.tile([64, HW], F32)
        nc.vector.tensor_copy(o_un[:], ps_o[0:64, :])
        s_row = r_pool.tile([1, HW], F32, tag="row")
        nc.vector.tensor_copy(s_row[:], ps_o[64:65, :])
        # normalize: O[d,i] * (1/s[i]); sums broadcast via DRAM bounce.
        r_row = r_pool.tile([1, HW], F32, tag="row")
        nc.vector.reciprocal_approx_fast(r_row[:], s_row[:])
        dr = dram.tile([1, HW], F32)
        nc.sync.dma_start(dr[:], r_row[:])
        rb = rb_pool.tile([64, HW], F32)
        nc.sync.dma_start(rb[:], dr[:].to_broadcast((64, HW)))
        use_gs = os.environ.get("K_NORM_GS", "1") == "1" and h < NH - 1
        norm_eng = nc.gpsimd if use_gs else nc.vector
        norm_eng.tensor_tensor(
            o_sb[po : po + 64, h // 2, :], o_un[:], rb[:], op=OP.mult
        )

    def outproj(b, o_sb, x_sb):
        for r in range(NC_CH):
            ps_y = ps_gp_pool.tile([128, HW], F32, tag="gp")
            for c in range(NC_CH):
                for half in range(2):
                    nc.tensor.matmul(
                        ps_y[:, half * 512 : (half + 1) * 512],
                        wto_sb[:, c, r * 128 : (r + 1) * 128],
                        o_sb[:, c, half * 512 : (half + 1) * 512],
                        start=(c == 0),
                        stop=(c == NC_CH - 1),
                    )
            y_t = y_pool.tile([128, HW], F32)
            nc.vector.scalar_tensor_tensor(
                out=y_t[:],
                in0=ps_y[:],
                scalar=bout_sb[:, r : r + 1],
                in1=x_sb[:, r, :],
                op0=OP.add,
                op1=OP.add,
            )
            nc.sync.dma_start(y_d[b, r * 128 : (r + 1) * 128, :], y_t[:])

    prev = None
    for b in range(NB):
        x_sb = load_x(b)
        xn_sb = xn_pool.tile([128, NC_CH, HW], F32R)
        qk_sb = qk_pool.tile([128, 8, HW], F32R)
        o_sb = o_pool.tile([128, NC_CH, HW], F32R)
        groupnorm(b, x_sb, xn_sb)
        v_transposed(b, xn_sb)
        for p in range(4):  # head pairs; qk chunks arrive just-in-time
            qk_chunk(b, xn_sb, qk_sb, p)
            qk_chunk(b, xn_sb, qk_sb, 4 + p)
            if p == 0 and prev is not None:
                outproj(*prev)
            attn_head(b, qk_sb, o_sb, 2 * p)
            attn_head(b, qk_sb, o_sb, 2 * p + 1)
        prev = (b, o_sb, x_sb)
    outproj(*prev)


_NC_CACHE = None


def _build():
    global _NC_CACHE
    if _NC_CACHE is not None:
        return _NC_CACHE
    import contextlib

    nc = bacc.Bacc("TRN2", target_bir_lowering=False, debug=False)
    with tile.TileContext(nc) as tc:
        with contextlib.ExitStack() as ctx:
            build_program(nc, tc, ctx)
    nc.compile()
    _NC_CACHE = nc
    return nc


def make_in_maps(x, gamma, beta, w_qkv, b_qkv, w_out, b_out):
    x = np.ascontiguousarray(np.asarray(x, dtype=np.float32))
    gamma = np.asarray(gamma, dtype=np.float32)
    beta = np.asarray(beta, dtype=np.float32)
    w_qkv = np.asarray(w_qkv, dtype=np.float32)
    b_qkv = np.asarray(b_qkv, dtype=np.float32)
    w_out = np.asarray(w_out, dtype=np.float32)
    b_out = np.asarray(b_out, dtype=np.float32)

    B, Cc, H, W = x.shape
    assert (B, Cc, H, W) == (16, 512, 32, 32)

    # host-side weight layout transforms (pure layout; no compute moved
    # off-device except the exact fold of the v-bias: softmax rows sum to 1,
    # so attn @ (v + b_v 1^T) = attn @ v + b_v, and W_out @ b_v folds into b_out)
    wt = np.ascontiguousarray(w_qkv.T)                      # [512, 1536]
    wto = np.ascontiguousarray(w_out.T)                     # [512, 512]
    b_out_eff = b_out + w_out @ b_qkv[2 * C : 3 * C]
    bq = np.ascontiguousarray(b_qkv[: 2 * C].reshape(8, 128).T)   # [128, 8]
    gam = np.ascontiguousarray(gamma.reshape(NC_CH, 128).T)       # [128, 4]
    bet = np.ascontiguousarray(beta.reshape(NC_CH, 128).T)
    bout = np.ascontiguousarray(b_out_eff.reshape(NC_CH, 128).T)

    gmask_np = np.zeros((128, NC_CH, 2 * NG), dtype=np.float32)
    gmaskT_np = np.zeros((2 * NG, NC_CH, 128), dtype=np.float32)
    for c in range(NC_CH):
        gmask_np[0:64, c, 2 * c] = 1.0
        gmask_np[64:128, c, 2 * c + 1] = 1.0
        gmaskT_np[2 * c, c, 0:64] = 1.0
        gmaskT_np[2 * c + 1, c, 64:128] = 1.0

    xr = x.reshape(16, 512, 1024)
    in_maps = []
    for core in range(8):
        in_maps.append(
            {
                "x": np.ascontiguousarray(xr[2 * core : 2 * core + 2]),
                "wt": wt,
                "wto": wto,
                "bq": bq,
                "gam": gam,
                "bet": bet,
                "bout": bout,
                "gmask": gmask_np,
                "gmaskT": gmaskT_np,
                "vtones": np.ones((128, NH, NH, 1), dtype=np.float32),
            }
        )
    return in_maps


def kernel(x, gamma, beta, w_qkv, b_qkv, w_out, b_out):
    in_maps = make_in_maps(x, gamma, beta, w_qkv, b_qkv, w_out, b_out)
    nc = _build()
    res = bass_utils.run_bass_kernel_spmd(nc, in_maps, core_ids=list(range(8)))
    out = np.concatenate([r["y"] for r in res.results], axis=0)
    return out.reshape(16, 512, 32, 32).astype(np.float32)
